# revision 15
# baseline (speedup 1.0000x reference)
"""BiLSTM-CRF Trainium2 Bass kernel. 8-core chunked-parallel design:

- T=4096 split into 8 chunks of 512, one per core (SPMD, same NEFF).
- Each core: gathers embeddings for its halo'd range U=[lo-256, hi+256),
  computes input projections xg = emb @ W_ih^T + b on PE, then runs both LSTM
  directions' recurrences over its chunk with burn-in (state errors contract;
  192-step burn-in => < 1e-10 state error, validated offline), computes
  feats^T = W_tag @ lstm_out^T, then two-sided Viterbi (alpha fwd + beta bwd
  max-plus scans, also with burn-in) and path[t] = argmax(alpha[t]+beta[t]).
- Sequence-edge cores are made uniform with interior cores via data:
  out-of-range embedding rows point at synthetic rows appended to the table
  that force gates i,f,o to sigmoid(-40)~=0 (so state is exactly ~0 entering
  t=0 / t=T-1), and feat spike columns pin the alpha/beta scan state to the
  true CRF boundary conditions up to an additive constant (invariant for the
  argmax).
- tanh(x) is computed as 2*sigmoid(2x)-1 (sigmoid and tanh live in different
  ACT table sets; alternating would reload tables every step). The 2x on the
  g-gate is folded into W_hh/W_ih/biases host-side.
- Score is assembled on the host from the decoded path and returned feats
  (a sum of 4097 table lookups).
"""
import numpy as np

T = 4096
K = 12
HID = 256
EMB = 300
EMBP = 384          # padded embedding row (3*128)
D = 4 * EMBP        # 1536 = 12 k-chunks of 128
VOCAB = 50000
NEG = -10000.0
START, STOP = 10, 11
CH = 512            # chunk per core
HB = 256            # lstm halo (burn-in 192 + 64 feats extension)
EXV = 64            # viterbi burn-in / feats extension
NU = CH + 2 * HB    # 1024 xg rows per core
NS = CH + HB + EXV  # 832 lstm steps per scan
NF = CH + 2 * EXV   # 640 kept lstm_out/feats rows
NV = CH + EXV       # 576 alpha steps
BIG = 10050.0

_CACHE = {}


def _build_program():
    import concourse.bass as bass
    import concourse.tile as tile
    import concourse.mybir as mybir
    from contextlib import ExitStack
    import tile_patch_k  # noqa: F401  (drain sync-wait splitting)

    f32 = mybir.dt.float32
    i32 = mybir.dt.int32
    SIG = mybir.ActivationFunctionType.Sigmoid

    nc = bass.Bass()
    dp = nc.declare_dram_parameter
    wemb = dp("wemb", [VOCAB + 8, EMBP], f32, isOutput=False)
    idx_f = dp("idx_f", [4, NU], i32, isOutput=False)
    whhfT = dp("whhfT", [256, 1024], f32, isOutput=False)
    whhbT = dp("whhbT", [256, 1024], f32, isOutput=False)
    wihfT = dp("wihfT", [D, 1024], f32, isOutput=False)
    wihbT = dp("wihbT", [D, 1024], f32, isOutput=False)
    bf8 = dp("bf8", [128, 8], f32, isOutput=False)
    bb8 = dp("bb8", [128, 8], f32, isOutput=False)
    wtagT = dp("wtagT", [512, 16], f32, isOutput=False)
    btag = dp("btag", [32, 1], f32, isOutput=False)
    transP = dp("transP", [32, 32], f32, isOutput=False)   # A padded -1e9
    transTP = dp("transTP", [32, 32], f32, isOutput=False) # A^T padded
    fixa = dp("fixa", [32, EXV], f32, isOutput=False)
    fixb = dp("fixb", [32, EXV], f32, isOutput=False)
    ident = dp("ident", [128, 128], f32, isOutput=False)
    iota12 = dp("iota12", [128, 16], f32, isOutput=False)
    path_o = dp("path_o", [128, 4], i32, isOutput=True)
    feats_o = dp("feats_o", [12, CH], f32, isOutput=True)

    with tile.TileContext(nc) as tc, ExitStack() as ctx:
        pool = ctx.enter_context(tc.tile_pool(name="main", bufs=1))
        psum = ctx.enter_context(tc.tile_pool(name="ps", bufs=2, space="PSUM"))
        tp = ctx.enter_context(tc.tile_pool(name="tmp", bufs=3))

        # ---- static tiles ----
        id_sb = pool.tile([128, 128], f32)
        nc.sync.dma_start(id_sb[:], ident[:])
        whh_sb = {}
        for dn, src in (("f", whhfT), ("b", whhbT)):
            for kc in range(2):
                t = pool.tile([128, 1024], f32, tag=f"whh{dn}{kc}")
                nc.sync.dma_start(t[:], src[kc * 128:(kc + 1) * 128, :])
                whh_sb[dn, kc] = t
        bias_sb = {}
        for dn, src in (("f", bf8), ("b", bb8)):
            t = pool.tile([128, 8], f32, tag=f"bias{dn}")
            nc.sync.dma_start(t[:], src[:])
            bias_sb[dn] = t
        wtag_sb = pool.tile([128, 4, 16], f32)
        nc.sync.dma_start(wtag_sb[:], wtagT[:].rearrange("(a p) k -> p a k", p=128))
        btag_sb = pool.tile([32, 1], f32)
        nc.sync.dma_start(btag_sb[:], btag[:])
        A_sb = pool.tile([32, 32], f32)
        nc.sync.dma_start(A_sb[:], transP[:])
        AT_sb = pool.tile([32, 32], f32)
        nc.sync.dma_start(AT_sb[:], transTP[:])
        fixa_sb = pool.tile([32, EXV], f32)
        nc.sync.dma_start(fixa_sb[:], fixa[:])
        fixb_sb = pool.tile([32, EXV], f32)
        nc.sync.dma_start(fixb_sb[:], fixb[:])
        iota_sb = pool.tile([128, 16], f32)
        nc.sync.dma_start(iota_sb[:], iota12[:])
        big_sb = pool.tile([128, 16], f32)
        nc.vector.memset(big_sb[:], 999.0)

        # ---- phase 1: gather embeddings ----
        # idx layout: stream s, NU entries. gather per 128-row tile.
        idx_sb = pool.tile([128, 32], i32)
        for s in range(4):
            for b in range(8):
                nc.sync.dma_start(
                    idx_sb[:, s * 8 + b: s * 8 + b + 1],
                    idx_f[s, b * 128:(b + 1) * 128].rearrange("(p a) -> p a", a=1))
        emb_g = {}
        for s in range(4):
            for b in range(8):
                g = tp.tile([128, EMBP], f32, tag="embg")
                nc.gpsimd.indirect_dma_start(
                    out=g[:], out_offset=None, in_=wemb[:],
                    in_offset=bass.IndirectOffsetOnAxis(
                        ap=idx_sb[:, s * 8 + b: s * 8 + b + 1], axis=0))
                emb_g[s, b] = g

        # ---- phase 2: transpose emb -> embT[kc] (128, NU), kc in 12 ----
        embT = [pool.tile([128, NU], f32, tag=f"embT{k}", name=f"embT{k}") for k in range(12)]
        for s in range(4):
            for b in range(8):
                for kb in range(3):
                    pt = psum.tile([128, 128], f32, tag="tp", bufs=2)
                    nc.tensor.transpose(
                        pt[:], emb_g[s, b][:, kb * 128:(kb + 1) * 128], id_sb[:])
                    nc.vector.tensor_copy(
                        embT[s * 3 + kb][:, b * 128:(b + 1) * 128], pt[:])

        # ---- phase 3: xg (t-major, chunk-minor layout) ----
        xg = {"f": pool.tile([128, NU, 8], f32, tag="xgf", name="xgf"),
              "b": pool.tile([128, NU, 8], f32, tag="xgb", name="xgb")}
        for dn, wih in (("f", wihfT), ("b", wihbT)):
            for m in range(8):
                wt = {}
                for kc in range(12):
                    w = tp.tile([128, 128], f32, tag="wih")
                    nc.sync.dma_start(
                        w[:], wih[kc * 128:(kc + 1) * 128, m * 128:(m + 1) * 128])
                    wt[kc] = w
                for tb in range(2):
                    pt = psum.tile([128, 512], f32, tag="xps", bufs=2)
                    for kc in range(12):
                        nc.tensor.matmul(
                            pt[:], wt[kc][:], embT[kc][:, tb * 512:(tb + 1) * 512],
                            start=(kc == 0), stop=(kc == 11))
                    nc.vector.tensor_scalar(
                        out=xg[dn][:, tb * 512:(tb + 1) * 512, m],
                        in0=pt[:], scalar1=bias_sb[dn][:, m:m + 1], scalar2=None,
                        op0=mybir.AluOpType.add)

        # ---- phase 4: LSTM scans (fwd + bwd interleaved) ----
        # hs_f: col 2(i+1)+j = h at scan step i (t = lo-256+i), j=half
        # hs_b: col 2i'+j with descending mapping => t-ascending layout.
        hs = {"f": pool.tile([128, 2 * NS + 4, ], f32, tag="hsf", name="hsf"),
              "b": pool.tile([128, 2 * NS + 4, ], f32, tag="hsb", name="hsb")}
        c_st = {"f": pool.tile([128, 2], f32, tag="cf", name="cf"),
                "b": pool.tile([128, 2], f32, tag="cb", name="cb")}
        hstage = {"f": pool.tile([128, 9, 2], f32, tag="hgf", name="hgf"),
                  "b": pool.tile([128, 9, 2], f32, tag="hgb", name="hgb")}
        xstage = {"f": pool.tile([128, 8, 8], f32, tag="xsf", name="xsf"),
                  "b": pool.tile([128, 8, 8], f32, tag="xsb", name="xsb")}
        for dn in ("f", "b"):
            nc.vector.memset(c_st[dn][:], 0.0)
            nc.vector.memset(hstage[dn][:], 0.0)

        UNROLL = 8
        assert NS % UNROLL == 0

        def lstm_step(dn, u):
            pg = psum.tile([128, 8], f32, tag=f"pg{dn}", bufs=1, name=f"pg{dn}")
            if dn == "f":
                rslot, wslot, xslot = u, u + 1, u
            else:
                rslot, wslot, xslot = 8 - u, 7 - u, 7 - u
            for m in range(8):
                for kc in range(2):
                    nc.tensor.matmul(
                        pg[:, m:m + 1], whh_sb[dn, kc][:, m * 128:(m + 1) * 128],
                        hstage[dn][:, rslot, kc:kc + 1],
                        start=(kc == 0), stop=(kc == 1))
            gt = tp.tile([128, 8], f32, tag=f"gt{dn}", name=f"gt{dn}")
            nc.vector.tensor_tensor(
                out=gt[:], in0=pg[:], in1=xstage[dn][:, xslot, :],
                op=mybir.AluOpType.add)
            s = tp.tile([128, 8], f32, tag=f"s{dn}", name=f"s{dn}")
            nc.scalar.activation(s[:], gt[:], SIG)
            g2 = tp.tile([128, 2], f32, tag=f"g2{dn}", name=f"g2{dn}")
            nc.vector.tensor_scalar(
                out=g2[:], in0=s[:, 4:6], scalar1=2.0, scalar2=-1.0,
                op0=mybir.AluOpType.mult, op1=mybir.AluOpType.add)
            t1 = tp.tile([128, 2], f32, tag=f"t1{dn}", name=f"t1{dn}")
            nc.vector.tensor_tensor(out=t1[:], in0=s[:, 0:2], in1=g2[:],
                                    op=mybir.AluOpType.mult)
            t2 = tp.tile([128, 2], f32, tag=f"t2{dn}", name=f"t2{dn}")
            nc.vector.tensor_tensor(out=t2[:], in0=s[:, 2:4], in1=c_st[dn][:],
                                    op=mybir.AluOpType.mult)
            nc.vector.tensor_tensor(out=c_st[dn][:], in0=t1[:], in1=t2[:],
                                    op=mybir.AluOpType.add)
            sc = tp.tile([128, 2], f32, tag=f"sc{dn}", name=f"sc{dn}")
            nc.scalar.activation(sc[:], c_st[dn][:], SIG, scale=2.0)
            m1 = tp.tile([128, 2], f32, tag=f"m1{dn}", name=f"m1{dn}")
            nc.vector.tensor_tensor(out=m1[:], in0=s[:, 6:8], in1=sc[:],
                                    op=mybir.AluOpType.mult)
            m2 = tp.tile([128, 2], f32, tag=f"m2{dn}", name=f"m2{dn}")
            nc.vector.tensor_scalar(out=m2[:], in0=m1[:], scalar1=2.0,
                                    scalar2=None, op0=mybir.AluOpType.mult)
            nc.vector.tensor_tensor(
                out=hstage[dn][:, wslot, :], in0=m2[:], in1=s[:, 6:8],
                op=mybir.AluOpType.subtract)

        with tc.For_i(0, NS, UNROLL) as iv:
            # stage xg blocks (one dynamic AP each)
            nc.vector.tensor_copy(xstage["f"][:], xg["f"][:, bass.ds(iv, 8), :])
            nc.vector.tensor_copy(
                xstage["b"][:], xg["b"][:, bass.ds(NU - 8 - iv, 8), :])
            for u in range(UNROLL):
                lstm_step("f", u)
                lstm_step("b", u)
            # flush h history blocks (one dynamic AP each) + carry slots
            nc.vector.tensor_copy(
                hs["f"][:, bass.ds(2 * iv + 2, 16)],
                hstage["f"][:, 1:9, :].rearrange("p a b -> p (a b)"))
            nc.vector.tensor_copy(
                hs["b"][:, bass.ds(2 * NS - 14 - 2 * iv, 16)],
                hstage["b"][:, 0:8, :].rearrange("p a b -> p (a b)"))
            nc.vector.tensor_copy(hstage["f"][:, 0, :], hstage["f"][:, 8, :])
            nc.vector.tensor_copy(hstage["b"][:, 8, :], hstage["b"][:, 0, :])

        # ---- phase 5: feats^T (32, NF) for t in [lo-64, hi+64) ----
        feats_sb = pool.tile([32, NF], f32)
        nc.vector.memset(feats_sb[:], 0.0)
        # hf(t): hs_f col 2*(t-lo+256)+2+j ; kept t range starts at lo-64
        # => scan step i0 = 192 => col base 2*192+2 = 386 (+j)
        # hb(t): hs_b col 2*NS-2*i+... t = hi+255-i ; t=lo-64 => i=831 => col 2.
        for nb in range(2):
            pf = psum.tile([12, 320], f32, tag="pvit", bufs=1, name="pf")
            for kc in range(4):
                dn = "f" if kc < 2 else "b"
                j = kc % 2
                base = (386 if dn == "f" else 2) + j + 640 * nb
                rhs = hs[dn][:, base: base + 640: 2]
                nc.tensor.matmul(pf[:], wtag_sb[:, kc, 0:12], rhs,
                                 start=(kc == 0), stop=(kc == 3))
            nc.vector.tensor_scalar(
                out=feats_sb[0:12, nb * 320:(nb + 1) * 320], in0=pf[:],
                scalar1=btag_sb[0:12, :], scalar2=None, op0=mybir.AluOpType.add)
        nc.vector.tensor_tensor(out=feats_sb[:, 0:EXV], in0=feats_sb[:, 0:EXV],
                                in1=fixa_sb[:], op=mybir.AluOpType.add)
        nc.vector.tensor_tensor(out=feats_sb[:, NF - EXV:NF],
                                in0=feats_sb[:, NF - EXV:NF],
                                in1=fixb_sb[:], op=mybir.AluOpType.add)

        # ---- phase 6: viterbi scans ----
        alphas = pool.tile([32, NV + 1], f32)
        betas = pool.tile([32, NF + 1], f32)
        nc.vector.memset(alphas[:, 0:1], 0.0)
        nc.vector.memset(betas[:, NF - 1:NF], 0.0)

        VUN_A = 16   # 576 = 16*36
        VUN_B = 23   # 575 = 23*25

        astage = pool.tile([32, VUN_A + 1], f32, tag="astage", name="astage")
        fstage_a = pool.tile([32, VUN_A], f32, tag="fsta", name="fsta")
        nc.vector.memset(astage[:], 0.0)
        with tc.For_i(0, NV, VUN_A) as iv:
            nc.gpsimd.tensor_copy(fstage_a[:], feats_sb[:, bass.ds(iv, VUN_A)])
            for u2 in range(VUN_A):
                ntvT = tp.tile([32, 32], f32, tag="ntvT", name="ntvT")
                nc.vector.tensor_scalar(
                    out=ntvT[:], in0=AT_sb[:], scalar1=astage[:, u2:u2 + 1],
                    scalar2=None, op0=mybir.AluOpType.add)
                ntv = tp.tile([32, 32], f32, tag="ntv", name="ntv")
                nc.vector.transpose(ntv[:], ntvT[:])
                mx = tp.tile([32, 1], f32, tag="mxa", name="mxa")
                nc.vector.reduce_max(mx[:], ntv[:], axis=mybir.AxisListType.X)
                nc.vector.tensor_tensor(
                    out=astage[:, u2 + 1:u2 + 2], in0=mx[:],
                    in1=fstage_a[:, u2:u2 + 1], op=mybir.AluOpType.add)
            nc.gpsimd.tensor_copy(alphas[:, bass.ds(iv + 1, VUN_A)],
                                  astage[:, 1:VUN_A + 1])
            nc.vector.tensor_copy(astage[:, 0:1], astage[:, VUN_A:VUN_A + 1])

        bstage = pool.tile([32, VUN_B + 1], f32, tag="bstage", name="bstage")
        fstage_b = pool.tile([32, VUN_B], f32, tag="fstb", name="fstb")
        nc.vector.memset(bstage[:], 0.0)
        with tc.For_i(0, NV - 1, VUN_B) as iv:
            nc.gpsimd.tensor_copy(fstage_b[:],
                                  feats_sb[:, bass.ds(NF - VUN_B - iv, VUN_B)])
            for u2 in range(VUN_B):
                ut = tp.tile([32, 1], f32, tag="ub", name="ub")
                nc.vector.tensor_tensor(
                    out=ut[:], in0=bstage[:, VUN_B - u2:VUN_B - u2 + 1],
                    in1=fstage_b[:, VUN_B - 1 - u2:VUN_B - u2],
                    op=mybir.AluOpType.add)
                M = tp.tile([32, 32], f32, tag="Mb", name="Mb")
                nc.vector.tensor_scalar(
                    out=M[:], in0=A_sb[:], scalar1=ut[:], scalar2=None,
                    op0=mybir.AluOpType.add)
                ptb = psum.tile([32, 32], f32, tag="pvit", bufs=1, name="ptb")
                nc.tensor.transpose(ptb[:], M[:], id_sb[0:32, 0:32])
                nc.vector.reduce_max(bstage[:, VUN_B - 1 - u2:VUN_B - u2],
                                     ptb[:], axis=mybir.AxisListType.X)
            nc.gpsimd.tensor_copy(betas[:, bass.ds(NF - 1 - VUN_B - iv, VUN_B)],
                                  bstage[:, 0:VUN_B])
            nc.vector.tensor_copy(bstage[:, VUN_B:VUN_B + 1], bstage[:, 0:1])

        # ---- phase 7: path = argmax(alpha+beta) over tags ----
        tot = pool.tile([32, CH], f32)
        nc.vector.tensor_tensor(out=tot[:], in0=alphas[:, EXV + 1: EXV + 1 + CH],
                                in1=betas[:, EXV: EXV + CH], op=mybir.AluOpType.add)
        path_sb = pool.tile([128, 4], i32)
        for b in range(4):
            ptp = psum.tile([128, 16], f32, tag="pvit", bufs=1, name="ptp")
            nc.tensor.transpose(ptp[:, 0:12], tot[0:12, b * 128:(b + 1) * 128],
                                id_sb[0:12, 0:12])
            totT = tp.tile([128, 16], f32, tag="totT")
            nc.vector.memset(totT[:], -3e9)
            nc.vector.tensor_copy(totT[:, 0:12], ptp[:, 0:12])
            mx = tp.tile([128, 1], f32, tag="mxp")
            nc.vector.reduce_max(mx[:], totT[:, 0:12], axis=mybir.AxisListType.X)
            msk = tp.tile([128, 16], i32, tag="msk")
            nc.vector.tensor_scalar(
                out=msk[:, 0:12], in0=totT[:, 0:12], scalar1=mx[:], scalar2=None,
                op0=mybir.AluOpType.is_equal)
            sel = tp.tile([128, 16], f32, tag="sel")
            nc.vector.select(sel[:, 0:12], msk[:, 0:12], iota_sb[:, 0:12],
                             big_sb[:, 0:12])
            idxf = tp.tile([128, 1], f32, tag="idxf")
            nc.vector.tensor_reduce(out=idxf[:], in_=sel[:, 0:12],
                                    op=mybir.AluOpType.min,
                                    axis=mybir.AxisListType.X)
            nc.vector.tensor_copy(path_sb[:, b:b + 1], idxf[:])
        nc.sync.dma_start(path_o[:], path_sb[:])
        nc.sync.dma_start(feats_o[:], feats_sb[0:12, EXV: EXV + CH])

    return nc


def _prep(inputs):
    """Host-side packing of per-core in_maps."""
    f = np.float32
    wf = {k: np.ascontiguousarray(np.asarray(v, dtype=f)) for k, v in inputs.items()
          if k not in ("words", "words1", "words2", "words3")}
    words = [np.asarray(inputs[k]).astype(np.int64) for k in
             ("words", "words1", "words2", "words3")]

    def scale_g(w_ih, w_hh, b):
        w_ih = w_ih.copy(); w_hh = w_hh.copy(); b = b.copy()
        w_ih[512:768] *= 2; w_hh[512:768] *= 2; b[512:768] *= 2
        return w_ih, w_hh, b

    out = {}
    for dn in ("f", "b"):
        b = wf[f"b_ih_{dn}"] + wf[f"b_hh_{dn}"]
        w_ih, w_hh, bsc = scale_g(wf[f"w_ih_{dn}"], wf[f"w_hh_{dn}"], b)
        # pad emb dim 300->384 per stream
        w_ih_p = np.zeros((1024, D), f)
        for s in range(4):
            w_ih_p[:, s * EMBP: s * EMBP + EMB] = w_ih[:, s * EMB:(s + 1) * EMB]
        out[f"wih{dn}T"] = np.ascontiguousarray(w_ih_p.T)
        out[f"whh{dn}T"] = np.ascontiguousarray(w_hh.T)
        out["bf8" if dn == "f" else "bb8"] = np.ascontiguousarray(
            bsc.reshape(8, 128).T)
        # special pad rows: solve w_ih_p @ e = target - b  (exact, min-norm)
        tgt = np.zeros(1024, np.float64)
        tgt[0:512] = -40.0; tgt[768:1024] = -40.0
        rhs = tgt - bsc.astype(np.float64)
        sol, res, rk, sv = np.linalg.lstsq(w_ih_p.astype(np.float64), rhs,
                                           rcond=None)
        out[f"espec_{dn}"] = sol.reshape(4, EMBP).astype(f)

    wemb_aug = np.zeros((VOCAB + 8, EMBP), f)
    wemb_aug[:VOCAB, :EMB] = wf["W_emb"]
    for dn, off in (("f", 0), ("b", 4)):
        wemb_aug[VOCAB + off: VOCAB + off + 4] = out[f"espec_{dn}"]

    wtagT = np.zeros((512, 16), f)
    wtagT[:, :12] = wf["W_tag"].T
    btag = np.zeros((32, 1), f)
    btag[:12, 0] = wf["b_tag"]
    trans = wf["transitions"]
    transP = np.full((32, 32), -1e9, f); transP[:12, :12] = trans
    transTP = np.full((32, 32), -1e9, f); transTP[:12, :12] = trans.T
    ident = np.eye(128, dtype=f)
    iota12 = np.zeros((128, 16), f)
    iota12[:, :12] = np.arange(12, dtype=f)[None, :]

    common = dict(wemb=wemb_aug, wtagT=wtagT, btag=btag, transP=transP,
                  transTP=transTP, ident=ident, iota12=iota12,
                  whhfT=out["whhfT"], whhbT=out["whhbT"],
                  wihfT=out["wihfT"], wihbT=out["wihbT"],
                  bf8=out["bf8"], bb8=out["bb8"])

    in_maps = []
    for m in range(8):
        lo = CH * m
        ts = np.arange(lo - HB, lo - HB + NU)
        idx = np.zeros((4, NU), np.int32)
        for s in range(4):
            v = np.where(ts < 0, VOCAB + s,
                         np.where(ts >= T, VOCAB + 4 + s, words[s][np.clip(ts, 0, T - 1)]))
            idx[s] = v.astype(np.int32)
        fixa = np.zeros((32, EXV), f)
        fixb = np.zeros((32, EXV), f)
        if m == 0:
            fixa[0:12, EXV - 1] = -BIG
            fixa[START, EXV - 1] = BIG
        if m == 7:
            fixb[0:12, 0] = -BIG
            fixb[STOP, 0] = BIG
        in_maps.append(dict(common, idx_f=idx, fixa=fixa, fixb=fixb))
    return in_maps, trans


def kernel(**inputs):
    _write_patch()
    from concourse.bass_utils import run_bass_kernel_spmd

    if "nc" not in _CACHE:
        _CACHE["nc"] = _build_program()
    nc = _CACHE["nc"]
    in_maps, trans = _prep(inputs)
    res = run_bass_kernel_spmd(nc, in_maps, list(range(8)))

    path = np.zeros(T, np.int64)
    feats = np.zeros((T, K), np.float32)
    for m in range(8):
        r = res.results[m]
        path[CH * m: CH * (m + 1)] = r["path_o"].T.reshape(-1)
        feats[CH * m: CH * (m + 1)] = r["feats_o"].T
    # host scoring
    sc = np.float64(trans[path[0], START]) + feats[0, path[0]]
    sc += (trans[path[1:], path[:-1]].astype(np.float64).sum()
           + feats[np.arange(1, T), path[1:]].astype(np.float64).sum())
    sc += trans[STOP, path[-1]]
    return np.float32(sc), path.astype(np.int32)


_PATCH_SRC = """
# Patch 1: TileContext final drain — split sync-waits (walrus here allows 1).
# Patch 2: Bass.to_json_bytes — split ANY multi-wait instruction in the BIR
# (loop back-edge drains, branches) into single-wait Drain chains.
import json
import concourse.tile as tile
import concourse.bass as bass_mod
from concourse.vector_clock import ScopedClock
from concourse import mybir

MAXW = 1

def _drain_and_barrier(self, tick_clock, wait_clock):
    drain_bi = self.nc.sync.drain()
    inst = drain_bi.ins
    wait_clock.add_sem_waits(inst, ScopedClock({None: tick_clock.global_clock}))
    si = inst.sync_info
    waits = list(si.on_wait) if si is not None else []
    if len(waits) > MAXW:
        si.on_wait = waits[:MAXW]
        inst.sync_info = si
        rest = waits[MAXW:]
        while rest:
            d2 = self.nc.sync.drain()
            d2.ins.sync_info = mybir.SyncInfo(on_wait=rest[:MAXW], on_update=[])
            rest = rest[MAXW:]
    self.nc.all_engine_barrier()
    assert self.sems is not None
    popped = self.nc._tile_sem_poison_stack.pop()
    assert popped is self._sem_poison
    self.nc.clear_and_free_semaphores(list(self.sems.allocated().values()))
    self.nc.all_engine_barrier()

tile.TileContext._drain_and_barrier = _drain_and_barrier

_orig_tjb = bass_mod.Bass.to_json_bytes

def _split_waits_json(self):
    m = json.loads(_orig_tjb(self))
    for fn in m["functions"]:
        for bb in fn["blocks"]:
            out = []
            for inst in bb["instructions"]:
                si = inst.get("sync_info")
                ws = si.get("on_wait") if si else None
                if ws and len(ws) > 1 and "engine" in inst:
                    for k, wt in enumerate(ws[:-1]):
                        d = {"engine": inst["engine"], "ins": [],
                             "name": inst["name"] + "-w%d" % k,
                             "opcode": "Drain", "outs": [],
                             "sync_info": {"on_update": [], "on_wait": [wt]}}
                        if "debug" in inst:
                            d["debug"] = inst["debug"]
                        out.append(d)
                    si["on_wait"] = [ws[-1]]
                out.append(inst)
            bb["instructions"] = out
    return json.dumps(m).encode()

bass_mod.Bass.to_json_bytes = _split_waits_json
"""


def _write_patch():
    import os
    p = "/tmp/tile_patch_k.py"
    with open(p, "w") as fh:
        fh.write(_PATCH_SRC)
    import sys
    if "/tmp" not in sys.path:
        sys.path.insert(0, "/tmp")


if __name__ == "__main__":
    d = np.load("/root/problem/inputs.npz")
    inp = {k: d[k] for k in d.files}
    s, p = kernel(**inp)
    r = np.load("/root/problem/ref_out.npz")
    print("score", s, "ref", r["score"])
    print("path mism:", int((p != r["path"]).sum()))


# revision 16
# speedup vs baseline: 289.0031x; 289.0031x over previous
"""BiLSTM-CRF Trainium2 Bass kernel. 8-core chunked-parallel design:

- T=4096 split into 8 chunks of 512, one per core (SPMD, same NEFF).
- Each core: gathers embeddings for its halo'd range U=[lo-256, hi+256),
  computes input projections xg = emb @ W_ih^T + b on PE, then runs both LSTM
  directions' recurrences over its chunk with burn-in (state errors contract;
  192-step burn-in => < 1e-10 state error, validated offline), computes
  feats^T = W_tag @ lstm_out^T, then two-sided Viterbi (alpha fwd + beta bwd
  max-plus scans, also with burn-in) and path[t] = argmax(alpha[t]+beta[t]).
- Sequence-edge cores are made uniform with interior cores via data:
  out-of-range embedding rows point at synthetic rows appended to the table
  that force gates i,f,o to sigmoid(-40)~=0 (so state is exactly ~0 entering
  t=0 / t=T-1), and feat spike columns pin the alpha/beta scan state to the
  true CRF boundary conditions up to an additive constant (invariant for the
  argmax).
- tanh(x) is computed as 2*sigmoid(2x)-1 (sigmoid and tanh live in different
  ACT table sets; alternating would reload tables every step). The 2x on the
  g-gate is folded into W_hh/W_ih/biases host-side.
- Score is assembled on the host from the decoded path and returned feats
  (a sum of 4097 table lookups).
"""
import numpy as np

T = 4096
K = 12
HID = 256
EMB = 300
EMBP = 384          # padded embedding row (3*128)
D = 4 * EMBP        # 1536 = 12 k-chunks of 128
VOCAB = 50000
NR = 4104        # per-core sliced table rows (4*NU + 8 specials)
NEG = -10000.0
START, STOP = 10, 11
CH = 512            # chunk per core
HB = 256            # lstm halo (burn-in 192 + 64 feats extension)
EXV = 64            # viterbi burn-in / feats extension
NU = CH + 2 * HB    # 1024 xg rows per core
NS = CH + HB + EXV  # 832 lstm steps per scan
NF = CH + 2 * EXV   # 640 kept lstm_out/feats rows
NV = CH + EXV       # 576 alpha steps
BIG = 10050.0

_CACHE = {}


def _build_program():
    import concourse.bass as bass
    import concourse.tile as tile
    import concourse.mybir as mybir
    from contextlib import ExitStack
    import tile_patch_k  # noqa: F401  (drain sync-wait splitting)

    f32 = mybir.dt.float32
    i32 = mybir.dt.int32
    SIG = mybir.ActivationFunctionType.Sigmoid

    nc = bass.Bass()
    dp = nc.declare_dram_parameter
    wemb = dp("wemb", [NR, EMBP], f32, isOutput=False)
    idx_f = dp("idx_f", [4, NU], i32, isOutput=False)
    whhfT = dp("whhfT", [256, 1024], f32, isOutput=False)
    whhbT = dp("whhbT", [256, 1024], f32, isOutput=False)
    wihfT = dp("wihfT", [D, 1024], f32, isOutput=False)
    wihbT = dp("wihbT", [D, 1024], f32, isOutput=False)
    bf8 = dp("bf8", [128, 8], f32, isOutput=False)
    bb8 = dp("bb8", [128, 8], f32, isOutput=False)
    wtagT = dp("wtagT", [512, 16], f32, isOutput=False)
    btag = dp("btag", [32, 1], f32, isOutput=False)
    transP = dp("transP", [32, 32], f32, isOutput=False)   # A padded -1e9
    transTP = dp("transTP", [32, 32], f32, isOutput=False) # A^T padded
    fixa = dp("fixa", [32, EXV], f32, isOutput=False)
    fixb = dp("fixb", [32, EXV], f32, isOutput=False)
    ident = dp("ident", [128, 128], f32, isOutput=False)
    iota12 = dp("iota12", [128, 16], f32, isOutput=False)
    path_o = dp("path_o", [128, 4], i32, isOutput=True)
    feats_o = dp("feats_o", [12, CH], f32, isOutput=True)

    with tile.TileContext(nc) as tc, ExitStack() as ctx:
        pool = ctx.enter_context(tc.tile_pool(name="main", bufs=1))
        psum = ctx.enter_context(tc.tile_pool(name="ps", bufs=2, space="PSUM"))
        tp = ctx.enter_context(tc.tile_pool(name="tmp", bufs=3))

        # ---- static tiles ----
        id_sb = pool.tile([128, 128], f32)
        nc.sync.dma_start(id_sb[:], ident[:])
        whh_sb = {}
        for dn, src in (("f", whhfT), ("b", whhbT)):
            for kc in range(2):
                t = pool.tile([128, 1024], f32, tag=f"whh{dn}{kc}")
                nc.sync.dma_start(t[:], src[kc * 128:(kc + 1) * 128, :])
                whh_sb[dn, kc] = t
        bias_sb = {}
        for dn, src in (("f", bf8), ("b", bb8)):
            t = pool.tile([128, 8], f32, tag=f"bias{dn}")
            nc.sync.dma_start(t[:], src[:])
            bias_sb[dn] = t
        wtag_sb = pool.tile([128, 4, 16], f32)
        nc.sync.dma_start(wtag_sb[:], wtagT[:].rearrange("(a p) k -> p a k", p=128))
        btag_sb = pool.tile([32, 1], f32)
        nc.sync.dma_start(btag_sb[:], btag[:])
        A_sb = pool.tile([32, 32], f32)
        nc.sync.dma_start(A_sb[:], transP[:])
        AT_sb = pool.tile([32, 32], f32)
        nc.sync.dma_start(AT_sb[:], transTP[:])
        fixa_sb = pool.tile([32, EXV], f32)
        nc.sync.dma_start(fixa_sb[:], fixa[:])
        fixb_sb = pool.tile([32, EXV], f32)
        nc.sync.dma_start(fixb_sb[:], fixb[:])
        iota_sb = pool.tile([128, 16], f32)
        nc.sync.dma_start(iota_sb[:], iota12[:])
        big_sb = pool.tile([128, 16], f32)
        nc.vector.memset(big_sb[:], 999.0)

        # ---- phase 1: gather embeddings ----
        # idx layout: stream s, NU entries. gather per 128-row tile.
        idx_sb = pool.tile([128, 32], i32)
        for s in range(4):
            for b in range(8):
                nc.sync.dma_start(
                    idx_sb[:, s * 8 + b: s * 8 + b + 1],
                    idx_f[s, b * 128:(b + 1) * 128].rearrange("(p a) -> p a", a=1))
        emb_g = {}
        for s in range(4):
            for b in range(8):
                g = tp.tile([128, EMBP], f32, tag="embg")
                nc.gpsimd.indirect_dma_start(
                    out=g[:], out_offset=None, in_=wemb[:],
                    in_offset=bass.IndirectOffsetOnAxis(
                        ap=idx_sb[:, s * 8 + b: s * 8 + b + 1], axis=0))
                emb_g[s, b] = g

        # ---- phase 2: transpose emb -> embT[kc] (128, NU), kc in 12 ----
        embT = [pool.tile([128, NU], f32, tag=f"embT{k}", name=f"embT{k}") for k in range(12)]
        for s in range(4):
            for b in range(8):
                for kb in range(3):
                    pt = psum.tile([128, 128], f32, tag="tp", bufs=2)
                    nc.tensor.transpose(
                        pt[:], emb_g[s, b][:, kb * 128:(kb + 1) * 128], id_sb[:])
                    nc.vector.tensor_copy(
                        embT[s * 3 + kb][:, b * 128:(b + 1) * 128], pt[:])

        # ---- phase 3: xg (t-major, chunk-minor layout) ----
        xg = {"f": pool.tile([128, NU, 8], f32, tag="xgf", name="xgf"),
              "b": pool.tile([128, NU, 8], f32, tag="xgb", name="xgb")}
        for dn, wih in (("f", wihfT), ("b", wihbT)):
            for m in range(8):
                wt = {}
                for kc in range(12):
                    w = tp.tile([128, 128], f32, tag="wih")
                    nc.sync.dma_start(
                        w[:], wih[kc * 128:(kc + 1) * 128, m * 128:(m + 1) * 128])
                    wt[kc] = w
                for tb in range(2):
                    pt = psum.tile([128, 512], f32, tag="xps", bufs=2)
                    for kc in range(12):
                        nc.tensor.matmul(
                            pt[:], wt[kc][:], embT[kc][:, tb * 512:(tb + 1) * 512],
                            start=(kc == 0), stop=(kc == 11))
                    nc.vector.tensor_scalar(
                        out=xg[dn][:, tb * 512:(tb + 1) * 512, m],
                        in0=pt[:], scalar1=bias_sb[dn][:, m:m + 1], scalar2=None,
                        op0=mybir.AluOpType.add)

        # ---- phase 4: LSTM scans (fwd + bwd interleaved) ----
        # hs_f: col 2(i+1)+j = h at scan step i (t = lo-256+i), j=half
        # hs_b: col 2i'+j with descending mapping => t-ascending layout.
        hs = {"f": pool.tile([128, 2 * NS + 4, ], f32, tag="hsf", name="hsf"),
              "b": pool.tile([128, 2 * NS + 4, ], f32, tag="hsb", name="hsb")}
        c_st = {"f": pool.tile([128, 2], f32, tag="cf", name="cf"),
                "b": pool.tile([128, 2], f32, tag="cb", name="cb")}
        hstage = {"f": pool.tile([128, 9, 2], f32, tag="hgf", name="hgf"),
                  "b": pool.tile([128, 9, 2], f32, tag="hgb", name="hgb")}
        xstage = {"f": pool.tile([128, 8, 8], f32, tag="xsf", name="xsf"),
                  "b": pool.tile([128, 8, 8], f32, tag="xsb", name="xsb")}
        for dn in ("f", "b"):
            nc.vector.memset(c_st[dn][:], 0.0)
            nc.vector.memset(hstage[dn][:], 0.0)

        UNROLL = 8
        assert NS % UNROLL == 0

        def lstm_step(dn, u):
            pg = psum.tile([128, 8], f32, tag=f"pg{dn}", bufs=1, name=f"pg{dn}")
            if dn == "f":
                rslot, wslot, xslot = u, u + 1, u
            else:
                rslot, wslot, xslot = 8 - u, 7 - u, 7 - u
            for m in range(8):
                for kc in range(2):
                    nc.tensor.matmul(
                        pg[:, m:m + 1], whh_sb[dn, kc][:, m * 128:(m + 1) * 128],
                        hstage[dn][:, rslot, kc:kc + 1],
                        start=(kc == 0), stop=(kc == 1))
            gt = tp.tile([128, 8], f32, tag=f"gt{dn}", name=f"gt{dn}")
            nc.vector.tensor_tensor(
                out=gt[:], in0=pg[:], in1=xstage[dn][:, xslot, :],
                op=mybir.AluOpType.add)
            s = tp.tile([128, 8], f32, tag=f"s{dn}", name=f"s{dn}")
            nc.scalar.activation(s[:], gt[:], SIG)
            g2 = tp.tile([128, 2], f32, tag=f"g2{dn}", name=f"g2{dn}")
            nc.vector.tensor_scalar(
                out=g2[:], in0=s[:, 4:6], scalar1=2.0, scalar2=-1.0,
                op0=mybir.AluOpType.mult, op1=mybir.AluOpType.add)
            t1 = tp.tile([128, 2], f32, tag=f"t1{dn}", name=f"t1{dn}")
            nc.vector.tensor_tensor(out=t1[:], in0=s[:, 0:2], in1=g2[:],
                                    op=mybir.AluOpType.mult)
            t2 = tp.tile([128, 2], f32, tag=f"t2{dn}", name=f"t2{dn}")
            nc.vector.tensor_tensor(out=t2[:], in0=s[:, 2:4], in1=c_st[dn][:],
                                    op=mybir.AluOpType.mult)
            nc.vector.tensor_tensor(out=c_st[dn][:], in0=t1[:], in1=t2[:],
                                    op=mybir.AluOpType.add)
            sc = tp.tile([128, 2], f32, tag=f"sc{dn}", name=f"sc{dn}")
            nc.scalar.activation(sc[:], c_st[dn][:], SIG, scale=2.0)
            m1 = tp.tile([128, 2], f32, tag=f"m1{dn}", name=f"m1{dn}")
            nc.vector.tensor_tensor(out=m1[:], in0=s[:, 6:8], in1=sc[:],
                                    op=mybir.AluOpType.mult)
            m2 = tp.tile([128, 2], f32, tag=f"m2{dn}", name=f"m2{dn}")
            nc.vector.tensor_scalar(out=m2[:], in0=m1[:], scalar1=2.0,
                                    scalar2=None, op0=mybir.AluOpType.mult)
            nc.vector.tensor_tensor(
                out=hstage[dn][:, wslot, :], in0=m2[:], in1=s[:, 6:8],
                op=mybir.AluOpType.subtract)

        with tc.For_i(0, NS, UNROLL) as iv:
            # stage xg blocks (one dynamic AP each)
            nc.vector.tensor_copy(xstage["f"][:], xg["f"][:, bass.ds(iv, 8), :])
            nc.vector.tensor_copy(
                xstage["b"][:], xg["b"][:, bass.ds(NU - 8 - iv, 8), :])
            for u in range(UNROLL):
                lstm_step("f", u)
                lstm_step("b", u)
            # flush h history blocks (one dynamic AP each) + carry slots
            nc.vector.tensor_copy(
                hs["f"][:, bass.ds(2 * iv + 2, 16)],
                hstage["f"][:, 1:9, :].rearrange("p a b -> p (a b)"))
            nc.vector.tensor_copy(
                hs["b"][:, bass.ds(2 * NS - 14 - 2 * iv, 16)],
                hstage["b"][:, 0:8, :].rearrange("p a b -> p (a b)"))
            nc.vector.tensor_copy(hstage["f"][:, 0, :], hstage["f"][:, 8, :])
            nc.vector.tensor_copy(hstage["b"][:, 8, :], hstage["b"][:, 0, :])

        # ---- phase 5: feats^T (32, NF) for t in [lo-64, hi+64) ----
        feats_sb = pool.tile([32, NF], f32)
        nc.vector.memset(feats_sb[:], 0.0)
        # hf(t): hs_f col 2*(t-lo+256)+2+j ; kept t range starts at lo-64
        # => scan step i0 = 192 => col base 2*192+2 = 386 (+j)
        # hb(t): hs_b col 2*NS-2*i+... t = hi+255-i ; t=lo-64 => i=831 => col 2.
        for nb in range(2):
            pf = psum.tile([12, 320], f32, tag="pvit", bufs=1, name="pf")
            for kc in range(4):
                dn = "f" if kc < 2 else "b"
                j = kc % 2
                base = (386 if dn == "f" else 2) + j + 640 * nb
                rhs = hs[dn][:, base: base + 640: 2]
                nc.tensor.matmul(pf[:], wtag_sb[:, kc, 0:12], rhs,
                                 start=(kc == 0), stop=(kc == 3))
            nc.vector.tensor_scalar(
                out=feats_sb[0:12, nb * 320:(nb + 1) * 320], in0=pf[:],
                scalar1=btag_sb[0:12, :], scalar2=None, op0=mybir.AluOpType.add)
        nc.vector.tensor_tensor(out=feats_sb[:, 0:EXV], in0=feats_sb[:, 0:EXV],
                                in1=fixa_sb[:], op=mybir.AluOpType.add)
        nc.vector.tensor_tensor(out=feats_sb[:, NF - EXV:NF],
                                in0=feats_sb[:, NF - EXV:NF],
                                in1=fixb_sb[:], op=mybir.AluOpType.add)

        # ---- phase 6: viterbi scans ----
        alphas = pool.tile([32, NV + 1], f32)
        betas = pool.tile([32, NF + 1], f32)
        nc.vector.memset(alphas[:, 0:1], 0.0)
        nc.vector.memset(betas[:, NF - 1:NF], 0.0)

        VUN_A = 16   # 576 = 16*36
        VUN_B = 23   # 575 = 23*25

        astage = pool.tile([32, VUN_A + 1], f32, tag="astage", name="astage")
        fstage_a = pool.tile([32, VUN_A], f32, tag="fsta", name="fsta")
        nc.vector.memset(astage[:], 0.0)
        with tc.For_i(0, NV, VUN_A) as iv:
            nc.gpsimd.tensor_copy(fstage_a[:], feats_sb[:, bass.ds(iv, VUN_A)])
            for u2 in range(VUN_A):
                ntvT = tp.tile([32, 32], f32, tag="ntvT", name="ntvT")
                nc.vector.tensor_scalar(
                    out=ntvT[:], in0=AT_sb[:], scalar1=astage[:, u2:u2 + 1],
                    scalar2=None, op0=mybir.AluOpType.add)
                ntv = tp.tile([32, 32], f32, tag="ntv", name="ntv")
                nc.vector.transpose(ntv[:], ntvT[:])
                mx = tp.tile([32, 1], f32, tag="mxa", name="mxa")
                nc.vector.reduce_max(mx[:], ntv[:], axis=mybir.AxisListType.X)
                nc.vector.tensor_tensor(
                    out=astage[:, u2 + 1:u2 + 2], in0=mx[:],
                    in1=fstage_a[:, u2:u2 + 1], op=mybir.AluOpType.add)
            nc.gpsimd.tensor_copy(alphas[:, bass.ds(iv + 1, VUN_A)],
                                  astage[:, 1:VUN_A + 1])
            nc.vector.tensor_copy(astage[:, 0:1], astage[:, VUN_A:VUN_A + 1])

        bstage = pool.tile([32, VUN_B + 1], f32, tag="bstage", name="bstage")
        fstage_b = pool.tile([32, VUN_B], f32, tag="fstb", name="fstb")
        nc.vector.memset(bstage[:], 0.0)
        with tc.For_i(0, NV - 1, VUN_B) as iv:
            nc.gpsimd.tensor_copy(fstage_b[:],
                                  feats_sb[:, bass.ds(NF - VUN_B - iv, VUN_B)])
            for u2 in range(VUN_B):
                ut = tp.tile([32, 1], f32, tag="ub", name="ub")
                nc.vector.tensor_tensor(
                    out=ut[:], in0=bstage[:, VUN_B - u2:VUN_B - u2 + 1],
                    in1=fstage_b[:, VUN_B - 1 - u2:VUN_B - u2],
                    op=mybir.AluOpType.add)
                M = tp.tile([32, 32], f32, tag="Mb", name="Mb")
                nc.vector.tensor_scalar(
                    out=M[:], in0=A_sb[:], scalar1=ut[:], scalar2=None,
                    op0=mybir.AluOpType.add)
                ptb = psum.tile([32, 32], f32, tag="pvit", bufs=1, name="ptb")
                nc.tensor.transpose(ptb[:], M[:], id_sb[0:32, 0:32])
                nc.vector.reduce_max(bstage[:, VUN_B - 1 - u2:VUN_B - u2],
                                     ptb[:], axis=mybir.AxisListType.X)
            nc.gpsimd.tensor_copy(betas[:, bass.ds(NF - 1 - VUN_B - iv, VUN_B)],
                                  bstage[:, 0:VUN_B])
            nc.vector.tensor_copy(bstage[:, VUN_B:VUN_B + 1], bstage[:, 0:1])

        # ---- phase 7: path = argmax(alpha+beta) over tags ----
        tot = pool.tile([32, CH], f32)
        nc.vector.tensor_tensor(out=tot[:], in0=alphas[:, EXV + 1: EXV + 1 + CH],
                                in1=betas[:, EXV: EXV + CH], op=mybir.AluOpType.add)
        path_sb = pool.tile([128, 4], i32)
        for b in range(4):
            ptp = psum.tile([128, 16], f32, tag="pvit", bufs=1, name="ptp")
            nc.tensor.transpose(ptp[:, 0:12], tot[0:12, b * 128:(b + 1) * 128],
                                id_sb[0:12, 0:12])
            totT = tp.tile([128, 16], f32, tag="totT")
            nc.vector.memset(totT[:], -3e9)
            nc.vector.tensor_copy(totT[:, 0:12], ptp[:, 0:12])
            mx = tp.tile([128, 1], f32, tag="mxp")
            nc.vector.reduce_max(mx[:], totT[:, 0:12], axis=mybir.AxisListType.X)
            msk = tp.tile([128, 16], i32, tag="msk")
            nc.vector.tensor_scalar(
                out=msk[:, 0:12], in0=totT[:, 0:12], scalar1=mx[:], scalar2=None,
                op0=mybir.AluOpType.is_equal)
            sel = tp.tile([128, 16], f32, tag="sel")
            nc.vector.select(sel[:, 0:12], msk[:, 0:12], iota_sb[:, 0:12],
                             big_sb[:, 0:12])
            idxf = tp.tile([128, 1], f32, tag="idxf")
            nc.vector.tensor_reduce(out=idxf[:], in_=sel[:, 0:12],
                                    op=mybir.AluOpType.min,
                                    axis=mybir.AxisListType.X)
            nc.vector.tensor_copy(path_sb[:, b:b + 1], idxf[:])
        nc.sync.dma_start(path_o[:], path_sb[:])
        nc.sync.dma_start(feats_o[:], feats_sb[0:12, EXV: EXV + CH])

    return nc


def _prep(inputs):
    """Host-side packing of per-core in_maps."""
    f = np.float32
    wf = {k: np.ascontiguousarray(np.asarray(v, dtype=f)) for k, v in inputs.items()
          if k not in ("words", "words1", "words2", "words3")}
    words = [np.asarray(inputs[k]).astype(np.int64) for k in
             ("words", "words1", "words2", "words3")]

    def scale_g(w_ih, w_hh, b):
        w_ih = w_ih.copy(); w_hh = w_hh.copy(); b = b.copy()
        w_ih[512:768] *= 2; w_hh[512:768] *= 2; b[512:768] *= 2
        return w_ih, w_hh, b

    out = {}
    for dn in ("f", "b"):
        b = wf[f"b_ih_{dn}"] + wf[f"b_hh_{dn}"]
        w_ih, w_hh, bsc = scale_g(wf[f"w_ih_{dn}"], wf[f"w_hh_{dn}"], b)
        # pad emb dim 300->384 per stream
        w_ih_p = np.zeros((1024, D), f)
        for s in range(4):
            w_ih_p[:, s * EMBP: s * EMBP + EMB] = w_ih[:, s * EMB:(s + 1) * EMB]
        out[f"wih{dn}T"] = np.ascontiguousarray(w_ih_p.T)
        out[f"whh{dn}T"] = np.ascontiguousarray(w_hh.T)
        out["bf8" if dn == "f" else "bb8"] = np.ascontiguousarray(
            bsc.reshape(8, 128).T)
        # special pad rows: solve w_ih_p @ e = target - b  (exact, min-norm)
        tgt = np.zeros(1024, np.float64)
        tgt[0:512] = -40.0; tgt[768:1024] = -40.0
        rhs = tgt - bsc.astype(np.float64)
        sol, res, rk, sv = np.linalg.lstsq(w_ih_p.astype(np.float64), rhs,
                                           rcond=None)
        out[f"espec_{dn}"] = sol.reshape(4, EMBP).astype(f)

    wemb_aug = np.zeros((VOCAB + 8, EMBP), f)
    wemb_aug[:VOCAB, :EMB] = wf["W_emb"]
    for dn, off in (("f", 0), ("b", 4)):
        wemb_aug[VOCAB + off: VOCAB + off + 4] = out[f"espec_{dn}"]

    wtagT = np.zeros((512, 16), f)
    wtagT[:, :12] = wf["W_tag"].T
    btag = np.zeros((32, 1), f)
    btag[:12, 0] = wf["b_tag"]
    trans = wf["transitions"]
    transP = np.full((32, 32), -1e9, f); transP[:12, :12] = trans
    transTP = np.full((32, 32), -1e9, f); transTP[:12, :12] = trans.T
    ident = np.eye(128, dtype=f)
    iota12 = np.zeros((128, 16), f)
    iota12[:, :12] = np.arange(12, dtype=f)[None, :]

    common = dict(wtagT=wtagT, btag=btag, transP=transP,
                  transTP=transTP, ident=ident, iota12=iota12,
                  whhfT=out["whhfT"], whhbT=out["whhbT"],
                  wihfT=out["wihfT"], wihbT=out["wihbT"],
                  bf8=out["bf8"], bb8=out["bb8"])

    in_maps = []
    for m in range(8):
        lo = CH * m
        ts = np.arange(lo - HB, lo - HB + NU)
        idx = np.zeros((4, NU), np.int32)
        for s in range(4):
            v = np.where(ts < 0, VOCAB + s,
                         np.where(ts >= T, VOCAB + 4 + s, words[s][np.clip(ts, 0, T - 1)]))
            idx[s] = v.astype(np.int32)
        used = np.unique(idx)
        remap_idx = np.searchsorted(used, idx).astype(np.int32)
        wemb_core = np.zeros((NR, EMBP), f)
        wemb_core[:len(used)] = wemb_aug[used]
        idx = remap_idx
        fixa = np.zeros((32, EXV), f)
        fixb = np.zeros((32, EXV), f)
        if m == 0:
            fixa[0:12, EXV - 1] = -BIG
            fixa[START, EXV - 1] = BIG
        if m == 7:
            fixb[0:12, 0] = -BIG
            fixb[STOP, 0] = BIG
        in_maps.append(dict(common, wemb=wemb_core, idx_f=idx, fixa=fixa, fixb=fixb))
    return in_maps, trans


def kernel(**inputs):
    _write_patch()
    from concourse.bass_utils import run_bass_kernel_spmd

    if "nc" not in _CACHE:
        _CACHE["nc"] = _build_program()
    nc = _CACHE["nc"]
    in_maps, trans = _prep(inputs)
    res = run_bass_kernel_spmd(nc, in_maps, list(range(8)))

    path = np.zeros(T, np.int64)
    feats = np.zeros((T, K), np.float32)
    for m in range(8):
        r = res.results[m]
        path[CH * m: CH * (m + 1)] = r["path_o"].T.reshape(-1)
        feats[CH * m: CH * (m + 1)] = r["feats_o"].T
    # host scoring
    sc = np.float64(trans[path[0], START]) + feats[0, path[0]]
    sc += (trans[path[1:], path[:-1]].astype(np.float64).sum()
           + feats[np.arange(1, T), path[1:]].astype(np.float64).sum())
    sc += trans[STOP, path[-1]]
    return np.float32(sc), path.astype(np.int32)


_PATCH_SRC = """
# Patch 1: TileContext final drain — split sync-waits (walrus here allows 1).
# Patch 2: Bass.to_json_bytes — split ANY multi-wait instruction in the BIR
# (loop back-edge drains, branches) into single-wait Drain chains.
import json
import concourse.tile as tile
import concourse.bass as bass_mod
from concourse.vector_clock import ScopedClock
from concourse import mybir

MAXW = 1

def _drain_and_barrier(self, tick_clock, wait_clock):
    drain_bi = self.nc.sync.drain()
    inst = drain_bi.ins
    wait_clock.add_sem_waits(inst, ScopedClock({None: tick_clock.global_clock}))
    si = inst.sync_info
    waits = list(si.on_wait) if si is not None else []
    if len(waits) > MAXW:
        si.on_wait = waits[:MAXW]
        inst.sync_info = si
        rest = waits[MAXW:]
        while rest:
            d2 = self.nc.sync.drain()
            d2.ins.sync_info = mybir.SyncInfo(on_wait=rest[:MAXW], on_update=[])
            rest = rest[MAXW:]
    self.nc.all_engine_barrier()
    assert self.sems is not None
    popped = self.nc._tile_sem_poison_stack.pop()
    assert popped is self._sem_poison
    self.nc.clear_and_free_semaphores(list(self.sems.allocated().values()))
    self.nc.all_engine_barrier()

tile.TileContext._drain_and_barrier = _drain_and_barrier

_orig_tjb = bass_mod.Bass.to_json_bytes

def _split_waits_json(self):
    m = json.loads(_orig_tjb(self))
    for fn in m["functions"]:
        for bb in fn["blocks"]:
            out = []
            for inst in bb["instructions"]:
                si = inst.get("sync_info")
                ws = si.get("on_wait") if si else None
                if ws and len(ws) > 1 and "engine" in inst:
                    for k, wt in enumerate(ws[:-1]):
                        d = {"engine": inst["engine"], "ins": [],
                             "name": inst["name"] + "-w%d" % k,
                             "opcode": "Drain", "outs": [],
                             "sync_info": {"on_update": [], "on_wait": [wt]}}
                        if "debug" in inst:
                            d["debug"] = inst["debug"]
                        out.append(d)
                    si["on_wait"] = [ws[-1]]
                out.append(inst)
            bb["instructions"] = out
    return json.dumps(m).encode()

bass_mod.Bass.to_json_bytes = _split_waits_json
"""


def _write_patch():
    import os
    p = "/tmp/tile_patch_k.py"
    with open(p, "w") as fh:
        fh.write(_PATCH_SRC)
    import sys
    if "/tmp" not in sys.path:
        sys.path.insert(0, "/tmp")


if __name__ == "__main__":
    d = np.load("/root/problem/inputs.npz")
    inp = {k: d[k] for k in d.files}
    s, p = kernel(**inp)
    r = np.load("/root/problem/ref_out.npz")
    print("score", s, "ref", r["score"])
    print("path mism:", int((p != r["path"]).sum()))


# revision 17
# speedup vs baseline: 415.3702x; 1.4373x over previous
"""BiLSTM-CRF Trainium2 Bass kernel. 8-core chunked-parallel design:

- T=4096 split into 8 chunks of 512, one per core (SPMD, same NEFF).
- Each core: gathers embeddings for its halo'd range U=[lo-256, hi+256),
  computes input projections xg = emb @ W_ih^T + b on PE, then runs both LSTM
  directions' recurrences over its chunk with burn-in (state errors contract;
  192-step burn-in => < 1e-10 state error, validated offline), computes
  feats^T = W_tag @ lstm_out^T, then two-sided Viterbi (alpha fwd + beta bwd
  max-plus scans, also with burn-in) and path[t] = argmax(alpha[t]+beta[t]).
- Sequence-edge cores are made uniform with interior cores via data:
  out-of-range embedding rows point at synthetic rows appended to the table
  that force gates i,f,o to sigmoid(-40)~=0 (so state is exactly ~0 entering
  t=0 / t=T-1), and feat spike columns pin the alpha/beta scan state to the
  true CRF boundary conditions up to an additive constant (invariant for the
  argmax).
- tanh(x) is computed as 2*sigmoid(2x)-1 (sigmoid and tanh live in different
  ACT table sets; alternating would reload tables every step). The 2x on the
  g-gate is folded into W_hh/W_ih/biases host-side.
- Score is assembled on the host from the decoded path and returned feats
  (a sum of 4097 table lookups).
"""
import numpy as np

T = 4096
K = 12
HID = 256
EMB = 300
EMBP = 384          # padded embedding row (3*128)
D = 4 * EMBP        # 1536 = 12 k-chunks of 128
VOCAB = 50000
NR = 4104        # per-core sliced table rows (4*NU + 8 specials)
NEG = -10000.0
START, STOP = 10, 11
CH = 512            # chunk per core
HB = 256            # lstm halo (burn-in 192 + 64 feats extension)
EXV = 64            # viterbi burn-in / feats extension
NU = CH + 2 * HB    # 1024 xg rows per core
NS = CH + HB + EXV  # 832 lstm steps per scan
NF = CH + 2 * EXV   # 640 kept lstm_out/feats rows
NV = CH + EXV       # 576 alpha steps
BIG = 10050.0

_CACHE = {}


def _build_program():
    import concourse.bass as bass
    import concourse.tile as tile
    import concourse.mybir as mybir
    from contextlib import ExitStack
    import tile_patch_k  # noqa: F401  (drain sync-wait splitting)

    f32 = mybir.dt.float32
    i32 = mybir.dt.int32
    SIG = mybir.ActivationFunctionType.Sigmoid

    nc = bass.Bass()
    dp = nc.declare_dram_parameter
    wemb = dp("wemb", [NR, EMBP], f32, isOutput=False)
    idx_f = dp("idx_f", [4, NU], i32, isOutput=False)
    whhfT = dp("whhfT", [256, 1024], f32, isOutput=False)
    whhbT = dp("whhbT", [256, 1024], f32, isOutput=False)
    wihfT = dp("wihfT", [D, 1024], f32, isOutput=False)
    wihbT = dp("wihbT", [D, 1024], f32, isOutput=False)
    bf8 = dp("bf8", [128, 8], f32, isOutput=False)
    bb8 = dp("bb8", [128, 8], f32, isOutput=False)
    wtagT = dp("wtagT", [512, 16], f32, isOutput=False)
    btag = dp("btag", [32, 1], f32, isOutput=False)
    transP = dp("transP", [32, 32], f32, isOutput=False)   # A padded -1e9
    transTP = dp("transTP", [32, 32], f32, isOutput=False) # A^T padded
    fixa = dp("fixa", [32, EXV], f32, isOutput=False)
    fixb = dp("fixb", [32, EXV], f32, isOutput=False)
    ident = dp("ident", [128, 128], f32, isOutput=False)
    iota12 = dp("iota12", [128, 16], f32, isOutput=False)
    path_o = dp("path_o", [128, 4], i32, isOutput=True)
    feats_o = dp("feats_o", [12, CH], f32, isOutput=True)

    with tile.TileContext(nc) as tc, ExitStack() as ctx:
        pool = ctx.enter_context(tc.tile_pool(name="main", bufs=1))
        psum = ctx.enter_context(tc.tile_pool(name="ps", bufs=2, space="PSUM"))
        tp = ctx.enter_context(tc.tile_pool(name="tmp", bufs=3))

        # ---- static tiles ----
        id_sb = pool.tile([128, 128], f32)
        nc.sync.dma_start(id_sb[:], ident[:])
        whh_sb = {}
        for dn, src in (("f", whhfT), ("b", whhbT)):
            for kc in range(2):
                t = pool.tile([128, 1024], f32, tag=f"whh{dn}{kc}")
                nc.sync.dma_start(t[:], src[kc * 128:(kc + 1) * 128, :])
                whh_sb[dn, kc] = t
        bias_sb = {}
        for dn, src in (("f", bf8), ("b", bb8)):
            t = pool.tile([128, 8], f32, tag=f"bias{dn}")
            nc.sync.dma_start(t[:], src[:])
            bias_sb[dn] = t
        wtag_sb = pool.tile([128, 4, 16], f32)
        nc.sync.dma_start(wtag_sb[:], wtagT[:].rearrange("(a p) k -> p a k", p=128))
        btag_sb = pool.tile([32, 1], f32)
        nc.sync.dma_start(btag_sb[:], btag[:])
        A_sb = pool.tile([32, 32], f32)
        nc.sync.dma_start(A_sb[:], transP[:])
        AT_sb = pool.tile([32, 32], f32)
        nc.sync.dma_start(AT_sb[:], transTP[:])
        fixa_sb = pool.tile([32, EXV], f32)
        nc.sync.dma_start(fixa_sb[:], fixa[:])
        fixb_sb = pool.tile([32, EXV], f32)
        nc.sync.dma_start(fixb_sb[:], fixb[:])
        iota_sb = pool.tile([128, 16], f32)
        nc.sync.dma_start(iota_sb[:], iota12[:])
        big_sb = pool.tile([128, 16], f32)
        nc.vector.memset(big_sb[:], 999.0)

        # ---- phase 1: gather embeddings ----
        # idx layout: stream s, NU entries. gather per 128-row tile.
        idx_sb = pool.tile([128, 32], i32)
        for s in range(4):
            for b in range(8):
                nc.sync.dma_start(
                    idx_sb[:, s * 8 + b: s * 8 + b + 1],
                    idx_f[s, b * 128:(b + 1) * 128].rearrange("(p a) -> p a", a=1))
        emb_g = {}
        for s in range(4):
            for b in range(8):
                g = tp.tile([128, EMBP], f32, tag="embg")
                nc.gpsimd.indirect_dma_start(
                    out=g[:], out_offset=None, in_=wemb[:],
                    in_offset=bass.IndirectOffsetOnAxis(
                        ap=idx_sb[:, s * 8 + b: s * 8 + b + 1], axis=0))
                emb_g[s, b] = g

        # ---- phase 2: transpose emb -> embT[kc] (128, NU), kc in 12 ----
        embT = [pool.tile([128, NU], f32, tag=f"embT{k}", name=f"embT{k}") for k in range(12)]
        for s in range(4):
            for b in range(8):
                for kb in range(3):
                    pt = psum.tile([128, 128], f32, tag="tp", bufs=2)
                    nc.tensor.transpose(
                        pt[:], emb_g[s, b][:, kb * 128:(kb + 1) * 128], id_sb[:])
                    nc.vector.tensor_copy(
                        embT[s * 3 + kb][:, b * 128:(b + 1) * 128], pt[:])

        # ---- phase 3: xg (t-major, chunk-minor layout) ----
        xg = {"f": pool.tile([128, NU, 8], f32, tag="xgf", name="xgf"),
              "b": pool.tile([128, NU, 8], f32, tag="xgb", name="xgb")}
        for dn, wih in (("f", wihfT), ("b", wihbT)):
            for m in range(8):
                wt = {}
                for kc in range(12):
                    w = tp.tile([128, 128], f32, tag="wih")
                    nc.sync.dma_start(
                        w[:], wih[kc * 128:(kc + 1) * 128, m * 128:(m + 1) * 128])
                    wt[kc] = w
                for tb in range(2):
                    pt = psum.tile([128, 512], f32, tag="xps", bufs=2)
                    for kc in range(12):
                        nc.tensor.matmul(
                            pt[:], wt[kc][:], embT[kc][:, tb * 512:(tb + 1) * 512],
                            start=(kc == 0), stop=(kc == 11))
                    nc.vector.tensor_scalar(
                        out=xg[dn][:, tb * 512:(tb + 1) * 512, m],
                        in0=pt[:], scalar1=bias_sb[dn][:, m:m + 1], scalar2=None,
                        op0=mybir.AluOpType.add)

        # ---- phase 4: LSTM scans (fwd + bwd interleaved) ----
        # hs_f: col 2(i+1)+j = h at scan step i (t = lo-256+i), j=half
        # hs_b: col 2i'+j with descending mapping => t-ascending layout.
        hs = {"f": pool.tile([128, 2 * NS + 4, ], f32, tag="hsf", name="hsf"),
              "b": pool.tile([128, 2 * NS + 4, ], f32, tag="hsb", name="hsb")}
        c_st = {"f": pool.tile([128, 2], f32, tag="cf", name="cf"),
                "b": pool.tile([128, 2], f32, tag="cb", name="cb")}
        UNROLL = 16
        hstage = {"f": pool.tile([128, UNROLL + 1, 2], f32, tag="hgf", name="hgf"),
                  "b": pool.tile([128, UNROLL + 1, 2], f32, tag="hgb", name="hgb")}
        xstage = {"f": pool.tile([128, UNROLL, 8], f32, tag="xsf", name="xsf"),
                  "b": pool.tile([128, UNROLL, 8], f32, tag="xsb", name="xsb")}
        for dn in ("f", "b"):
            nc.vector.memset(c_st[dn][:], 0.0)
            nc.vector.memset(hstage[dn][:], 0.0)

        assert NS % UNROLL == 0

        def lstm_step(dn, u):
            pg = psum.tile([128, 8], f32, tag=f"pg{dn}", bufs=1, name=f"pg{dn}")
            if dn == "f":
                rslot, wslot, xslot = u, u + 1, u
            else:
                rslot, wslot, xslot = UNROLL - u, UNROLL - 1 - u, UNROLL - 1 - u
            for m in range(8):
                for kc in range(2):
                    nc.tensor.matmul(
                        pg[:, m:m + 1], whh_sb[dn, kc][:, m * 128:(m + 1) * 128],
                        hstage[dn][:, rslot, kc:kc + 1],
                        start=(kc == 0), stop=(kc == 1))
            gt = tp.tile([128, 8], f32, tag=f"gt{dn}", name=f"gt{dn}")
            nc.vector.tensor_tensor(
                out=gt[:], in0=pg[:], in1=xstage[dn][:, xslot, :],
                op=mybir.AluOpType.add)
            s = tp.tile([128, 8], f32, tag=f"s{dn}", name=f"s{dn}")
            nc.scalar.activation(s[:], gt[:], SIG)
            g2 = tp.tile([128, 2], f32, tag=f"g2{dn}", name=f"g2{dn}")
            nc.vector.tensor_scalar(
                out=g2[:], in0=s[:, 4:6], scalar1=2.0, scalar2=-1.0,
                op0=mybir.AluOpType.mult, op1=mybir.AluOpType.add)
            t1 = tp.tile([128, 2], f32, tag=f"t1{dn}", name=f"t1{dn}")
            nc.vector.tensor_tensor(out=t1[:], in0=s[:, 0:2], in1=g2[:],
                                    op=mybir.AluOpType.mult)
            t2 = tp.tile([128, 2], f32, tag=f"t2{dn}", name=f"t2{dn}")
            nc.vector.tensor_tensor(out=t2[:], in0=s[:, 2:4], in1=c_st[dn][:],
                                    op=mybir.AluOpType.mult)
            nc.vector.tensor_tensor(out=c_st[dn][:], in0=t1[:], in1=t2[:],
                                    op=mybir.AluOpType.add)
            sc = tp.tile([128, 2], f32, tag=f"sc{dn}", name=f"sc{dn}")
            nc.scalar.activation(sc[:], c_st[dn][:], SIG, scale=2.0)
            m1 = tp.tile([128, 2], f32, tag=f"m1{dn}", name=f"m1{dn}")
            nc.vector.tensor_tensor(out=m1[:], in0=s[:, 6:8], in1=sc[:],
                                    op=mybir.AluOpType.mult)
            m2 = tp.tile([128, 2], f32, tag=f"m2{dn}", name=f"m2{dn}")
            nc.vector.tensor_scalar(out=m2[:], in0=m1[:], scalar1=2.0,
                                    scalar2=None, op0=mybir.AluOpType.mult)
            nc.vector.tensor_tensor(
                out=hstage[dn][:, wslot, :], in0=m2[:], in1=s[:, 6:8],
                op=mybir.AluOpType.subtract)

        with tc.For_i(0, NS, UNROLL, hint_engines=(mybir.EngineType.PE,)) as iv:
            # stage xg blocks (one dynamic AP each)
            nc.vector.tensor_copy(xstage["f"][:], xg["f"][:, bass.ds(iv, UNROLL), :])
            nc.vector.tensor_copy(
                xstage["b"][:], xg["b"][:, bass.ds(NU - UNROLL - iv, UNROLL), :])
            for u in range(UNROLL):
                lstm_step("f", u)
                lstm_step("b", u)
            # flush h history blocks (one dynamic AP each) + carry slots
            nc.vector.tensor_copy(
                hs["f"][:, bass.ds(2 * iv + 2, 2 * UNROLL)],
                hstage["f"][:, 1:UNROLL + 1, :].rearrange("p a b -> p (a b)"))
            nc.vector.tensor_copy(
                hs["b"][:, bass.ds(2 * NS - (2 * UNROLL - 2) - 2 * iv, 2 * UNROLL)],
                hstage["b"][:, 0:UNROLL, :].rearrange("p a b -> p (a b)"))
            nc.vector.tensor_copy(hstage["f"][:, 0, :], hstage["f"][:, UNROLL, :])
            nc.vector.tensor_copy(hstage["b"][:, UNROLL, :], hstage["b"][:, 0, :])

        # ---- phase 5: feats^T (32, NF) for t in [lo-64, hi+64) ----
        feats_sb = pool.tile([32, NF], f32)
        nc.vector.memset(feats_sb[:], 0.0)
        # hf(t): hs_f col 2*(t-lo+256)+2+j ; kept t range starts at lo-64
        # => scan step i0 = 192 => col base 2*192+2 = 386 (+j)
        # hb(t): hs_b col 2*NS-2*i+... t = hi+255-i ; t=lo-64 => i=831 => col 2.
        for nb in range(2):
            pf = psum.tile([12, 320], f32, tag="pvit", bufs=1, name="pf")
            for kc in range(4):
                dn = "f" if kc < 2 else "b"
                j = kc % 2
                base = (386 if dn == "f" else 2) + j + 640 * nb
                rhs = hs[dn][:, base: base + 640: 2]
                nc.tensor.matmul(pf[:], wtag_sb[:, kc, 0:12], rhs,
                                 start=(kc == 0), stop=(kc == 3))
            nc.vector.tensor_scalar(
                out=feats_sb[0:12, nb * 320:(nb + 1) * 320], in0=pf[:],
                scalar1=btag_sb[0:12, :], scalar2=None, op0=mybir.AluOpType.add)
        nc.vector.tensor_tensor(out=feats_sb[:, 0:EXV], in0=feats_sb[:, 0:EXV],
                                in1=fixa_sb[:], op=mybir.AluOpType.add)
        nc.vector.tensor_tensor(out=feats_sb[:, NF - EXV:NF],
                                in0=feats_sb[:, NF - EXV:NF],
                                in1=fixb_sb[:], op=mybir.AluOpType.add)

        # ---- phase 6: viterbi scans ----
        alphas = pool.tile([32, NV + 1], f32)
        betas = pool.tile([32, NF + 1], f32)
        nc.vector.memset(alphas[:, 0:1], 0.0)
        nc.vector.memset(betas[:, NF - 1:NF], 0.0)

        VUN_A = 16   # 576 = 16*36
        VUN_B = 23   # 575 = 23*25

        astage = pool.tile([32, VUN_A + 1], f32, tag="astage", name="astage")
        fstage_a = pool.tile([32, VUN_A], f32, tag="fsta", name="fsta")
        nc.vector.memset(astage[:], 0.0)
        with tc.For_i(0, NV, VUN_A) as iv:
            nc.gpsimd.tensor_copy(fstage_a[:], feats_sb[:, bass.ds(iv, VUN_A)])
            for u2 in range(VUN_A):
                ntvT = tp.tile([32, 32], f32, tag="ntvT", name="ntvT")
                nc.vector.tensor_scalar(
                    out=ntvT[:], in0=AT_sb[:], scalar1=astage[:, u2:u2 + 1],
                    scalar2=None, op0=mybir.AluOpType.add)
                ntv = tp.tile([32, 32], f32, tag="ntv", name="ntv")
                nc.vector.transpose(ntv[:], ntvT[:])
                mx = tp.tile([32, 1], f32, tag="mxa", name="mxa")
                nc.vector.reduce_max(mx[:], ntv[:], axis=mybir.AxisListType.X)
                nc.vector.tensor_tensor(
                    out=astage[:, u2 + 1:u2 + 2], in0=mx[:],
                    in1=fstage_a[:, u2:u2 + 1], op=mybir.AluOpType.add)
            nc.gpsimd.tensor_copy(alphas[:, bass.ds(iv + 1, VUN_A)],
                                  astage[:, 1:VUN_A + 1])
            nc.vector.tensor_copy(astage[:, 0:1], astage[:, VUN_A:VUN_A + 1])

        bstage = pool.tile([32, VUN_B + 1], f32, tag="bstage", name="bstage")
        fstage_b = pool.tile([32, VUN_B], f32, tag="fstb", name="fstb")
        nc.vector.memset(bstage[:], 0.0)
        with tc.For_i(0, NV - 1, VUN_B) as iv:
            nc.gpsimd.tensor_copy(fstage_b[:],
                                  feats_sb[:, bass.ds(NF - VUN_B - iv, VUN_B)])
            for u2 in range(VUN_B):
                ut = tp.tile([32, 1], f32, tag="ub", name="ub")
                nc.vector.tensor_tensor(
                    out=ut[:], in0=bstage[:, VUN_B - u2:VUN_B - u2 + 1],
                    in1=fstage_b[:, VUN_B - 1 - u2:VUN_B - u2],
                    op=mybir.AluOpType.add)
                M = tp.tile([32, 32], f32, tag="Mb", name="Mb")
                nc.vector.tensor_scalar(
                    out=M[:], in0=A_sb[:], scalar1=ut[:], scalar2=None,
                    op0=mybir.AluOpType.add)
                ptb = psum.tile([32, 32], f32, tag="pvit", bufs=1, name="ptb")
                nc.tensor.transpose(ptb[:], M[:], id_sb[0:32, 0:32])
                nc.vector.reduce_max(bstage[:, VUN_B - 1 - u2:VUN_B - u2],
                                     ptb[:], axis=mybir.AxisListType.X)
            nc.gpsimd.tensor_copy(betas[:, bass.ds(NF - 1 - VUN_B - iv, VUN_B)],
                                  bstage[:, 0:VUN_B])
            nc.vector.tensor_copy(bstage[:, VUN_B:VUN_B + 1], bstage[:, 0:1])

        # ---- phase 7: path = argmax(alpha+beta) over tags ----
        tot = pool.tile([32, CH], f32)
        nc.vector.tensor_tensor(out=tot[:], in0=alphas[:, EXV + 1: EXV + 1 + CH],
                                in1=betas[:, EXV: EXV + CH], op=mybir.AluOpType.add)
        path_sb = pool.tile([128, 4], i32)
        for b in range(4):
            ptp = psum.tile([128, 16], f32, tag="pvit", bufs=1, name="ptp")
            nc.tensor.transpose(ptp[:, 0:12], tot[0:12, b * 128:(b + 1) * 128],
                                id_sb[0:12, 0:12])
            totT = tp.tile([128, 16], f32, tag="totT")
            nc.vector.memset(totT[:], -3e9)
            nc.vector.tensor_copy(totT[:, 0:12], ptp[:, 0:12])
            mx = tp.tile([128, 1], f32, tag="mxp")
            nc.vector.reduce_max(mx[:], totT[:, 0:12], axis=mybir.AxisListType.X)
            msk = tp.tile([128, 16], i32, tag="msk")
            nc.vector.tensor_scalar(
                out=msk[:, 0:12], in0=totT[:, 0:12], scalar1=mx[:], scalar2=None,
                op0=mybir.AluOpType.is_equal)
            sel = tp.tile([128, 16], f32, tag="sel")
            nc.vector.select(sel[:, 0:12], msk[:, 0:12], iota_sb[:, 0:12],
                             big_sb[:, 0:12])
            idxf = tp.tile([128, 1], f32, tag="idxf")
            nc.vector.tensor_reduce(out=idxf[:], in_=sel[:, 0:12],
                                    op=mybir.AluOpType.min,
                                    axis=mybir.AxisListType.X)
            nc.vector.tensor_copy(path_sb[:, b:b + 1], idxf[:])
        nc.sync.dma_start(path_o[:], path_sb[:])
        nc.sync.dma_start(feats_o[:], feats_sb[0:12, EXV: EXV + CH])

    return nc


def _prep(inputs):
    """Host-side packing of per-core in_maps."""
    f = np.float32
    wf = {k: np.ascontiguousarray(np.asarray(v, dtype=f)) for k, v in inputs.items()
          if k not in ("words", "words1", "words2", "words3")}
    words = [np.asarray(inputs[k]).astype(np.int64) for k in
             ("words", "words1", "words2", "words3")]

    def scale_g(w_ih, w_hh, b):
        w_ih = w_ih.copy(); w_hh = w_hh.copy(); b = b.copy()
        w_ih[512:768] *= 2; w_hh[512:768] *= 2; b[512:768] *= 2
        return w_ih, w_hh, b

    out = {}
    for dn in ("f", "b"):
        b = wf[f"b_ih_{dn}"] + wf[f"b_hh_{dn}"]
        w_ih, w_hh, bsc = scale_g(wf[f"w_ih_{dn}"], wf[f"w_hh_{dn}"], b)
        # pad emb dim 300->384 per stream
        w_ih_p = np.zeros((1024, D), f)
        for s in range(4):
            w_ih_p[:, s * EMBP: s * EMBP + EMB] = w_ih[:, s * EMB:(s + 1) * EMB]
        out[f"wih{dn}T"] = np.ascontiguousarray(w_ih_p.T)
        out[f"whh{dn}T"] = np.ascontiguousarray(w_hh.T)
        out["bf8" if dn == "f" else "bb8"] = np.ascontiguousarray(
            bsc.reshape(8, 128).T)
        # special pad rows: solve w_ih_p @ e = target - b  (exact, min-norm)
        tgt = np.zeros(1024, np.float64)
        tgt[0:512] = -40.0; tgt[768:1024] = -40.0
        rhs = tgt - bsc.astype(np.float64)
        sol, res, rk, sv = np.linalg.lstsq(w_ih_p.astype(np.float64), rhs,
                                           rcond=None)
        out[f"espec_{dn}"] = sol.reshape(4, EMBP).astype(f)

    wemb_aug = np.zeros((VOCAB + 8, EMBP), f)
    wemb_aug[:VOCAB, :EMB] = wf["W_emb"]
    for dn, off in (("f", 0), ("b", 4)):
        wemb_aug[VOCAB + off: VOCAB + off + 4] = out[f"espec_{dn}"]

    wtagT = np.zeros((512, 16), f)
    wtagT[:, :12] = wf["W_tag"].T
    btag = np.zeros((32, 1), f)
    btag[:12, 0] = wf["b_tag"]
    trans = wf["transitions"]
    transP = np.full((32, 32), -1e9, f); transP[:12, :12] = trans
    transTP = np.full((32, 32), -1e9, f); transTP[:12, :12] = trans.T
    ident = np.eye(128, dtype=f)
    iota12 = np.zeros((128, 16), f)
    iota12[:, :12] = np.arange(12, dtype=f)[None, :]

    common = dict(wtagT=wtagT, btag=btag, transP=transP,
                  transTP=transTP, ident=ident, iota12=iota12,
                  whhfT=out["whhfT"], whhbT=out["whhbT"],
                  wihfT=out["wihfT"], wihbT=out["wihbT"],
                  bf8=out["bf8"], bb8=out["bb8"])

    in_maps = []
    for m in range(8):
        lo = CH * m
        ts = np.arange(lo - HB, lo - HB + NU)
        idx = np.zeros((4, NU), np.int32)
        for s in range(4):
            v = np.where(ts < 0, VOCAB + s,
                         np.where(ts >= T, VOCAB + 4 + s, words[s][np.clip(ts, 0, T - 1)]))
            idx[s] = v.astype(np.int32)
        used = np.unique(idx)
        remap_idx = np.searchsorted(used, idx).astype(np.int32)
        wemb_core = np.zeros((NR, EMBP), f)
        wemb_core[:len(used)] = wemb_aug[used]
        idx = remap_idx
        fixa = np.zeros((32, EXV), f)
        fixb = np.zeros((32, EXV), f)
        if m == 0:
            fixa[0:12, EXV - 1] = -BIG
            fixa[START, EXV - 1] = BIG
        if m == 7:
            fixb[0:12, 0] = -BIG
            fixb[STOP, 0] = BIG
        in_maps.append(dict(common, wemb=wemb_core, idx_f=idx, fixa=fixa, fixb=fixb))
    return in_maps, trans


def kernel(**inputs):
    _write_patch()
    from concourse.bass_utils import run_bass_kernel_spmd

    if "nc" not in _CACHE:
        _CACHE["nc"] = _build_program()
    nc = _CACHE["nc"]
    in_maps, trans = _prep(inputs)
    res = run_bass_kernel_spmd(nc, in_maps, list(range(8)))

    path = np.zeros(T, np.int64)
    feats = np.zeros((T, K), np.float32)
    for m in range(8):
        r = res.results[m]
        path[CH * m: CH * (m + 1)] = r["path_o"].T.reshape(-1)
        feats[CH * m: CH * (m + 1)] = r["feats_o"].T
    # host scoring
    sc = np.float64(trans[path[0], START]) + feats[0, path[0]]
    sc += (trans[path[1:], path[:-1]].astype(np.float64).sum()
           + feats[np.arange(1, T), path[1:]].astype(np.float64).sum())
    sc += trans[STOP, path[-1]]
    return np.float32(sc), path.astype(np.int32)


_PATCH_SRC = """
# Patch 1: TileContext final drain — split sync-waits (walrus here allows 1).
# Patch 2: Bass.to_json_bytes — split ANY multi-wait instruction in the BIR
# (loop back-edge drains, branches) into single-wait Drain chains.
import json
import concourse.tile as tile
import concourse.bass as bass_mod
from concourse.vector_clock import ScopedClock
from concourse import mybir

MAXW = 1

def _drain_and_barrier(self, tick_clock, wait_clock):
    drain_bi = self.nc.sync.drain()
    inst = drain_bi.ins
    wait_clock.add_sem_waits(inst, ScopedClock({None: tick_clock.global_clock}))
    si = inst.sync_info
    waits = list(si.on_wait) if si is not None else []
    if len(waits) > MAXW:
        si.on_wait = waits[:MAXW]
        inst.sync_info = si
        rest = waits[MAXW:]
        while rest:
            d2 = self.nc.sync.drain()
            d2.ins.sync_info = mybir.SyncInfo(on_wait=rest[:MAXW], on_update=[])
            rest = rest[MAXW:]
    self.nc.all_engine_barrier()
    assert self.sems is not None
    popped = self.nc._tile_sem_poison_stack.pop()
    assert popped is self._sem_poison
    self.nc.clear_and_free_semaphores(list(self.sems.allocated().values()))
    self.nc.all_engine_barrier()

tile.TileContext._drain_and_barrier = _drain_and_barrier

_orig_tjb = bass_mod.Bass.to_json_bytes

def _split_waits_json(self):
    m = json.loads(_orig_tjb(self))
    for fn in m["functions"]:
        for bb in fn["blocks"]:
            out = []
            for inst in bb["instructions"]:
                si = inst.get("sync_info")
                ws = si.get("on_wait") if si else None
                if ws and len(ws) > 1 and "engine" in inst:
                    for k, wt in enumerate(ws[:-1]):
                        d = {"engine": inst["engine"], "ins": [],
                             "name": inst["name"] + "-w%d" % k,
                             "opcode": "Drain", "outs": [],
                             "sync_info": {"on_update": [], "on_wait": [wt]}}
                        if "debug" in inst:
                            d["debug"] = inst["debug"]
                        out.append(d)
                    si["on_wait"] = [ws[-1]]
                out.append(inst)
            bb["instructions"] = out
    return json.dumps(m).encode()

bass_mod.Bass.to_json_bytes = _split_waits_json
"""


def _write_patch():
    import os
    p = "/tmp/tile_patch_k.py"
    with open(p, "w") as fh:
        fh.write(_PATCH_SRC)
    import sys
    if "/tmp" not in sys.path:
        sys.path.insert(0, "/tmp")


if __name__ == "__main__":
    d = np.load("/root/problem/inputs.npz")
    inp = {k: d[k] for k in d.files}
    s, p = kernel(**inp)
    r = np.load("/root/problem/ref_out.npz")
    print("score", s, "ref", r["score"])
    print("path mism:", int((p != r["path"]).sum()))
